# revision 1
# baseline (speedup 1.0000x reference)
"""Trainium2 Bass kernel for nn_DiagnosticRNN (LSTM B=2048,T=128,V=25,H=512
-> FC 100), 8-way batch-data-parallel across NeuronCores.

Strategy
--------
Data-parallel over batch: each of the 8 cores runs the full T=128 LSTM
recurrence on BS=256 batch rows with all weights replicated (per the
sharding hint). Everything is fused on-chip; the naive approach's
[B,T,4H] x-projection (2.1 GB of DRAM traffic) is never materialized.

Per-core per-timestep (all matmul operands bf16, fp32 PSUM accumulate):
  gates[4H, BS] = W_hh_perm @ h_{t-1} + W_ihaug_perm @ [x_t; 1]
    * 16 M-tiles (permuted gate order: m = 4j+q, q in (i,f,g,o), j the
      h-chunk), N=BS=256, PSUM bank b holds M-tile pair (2b, 2b+1).
    * x-term: thin K=26 matmuls (V=25 + a ones row folding b_ih+b_hh),
      4-way row-tiled via tile_position (x replicated at partition
      offsets 0/32/64/96); even m start=True then odd m start=False so
      each PSUM bank's has_written clear happens exactly once per step.
    * W_hh term: 64 MMs, k-outer so step t+1's k-th sweep only needs
      h-chunk k -> deep cross-step pipelining with ACT/DVE.
  ScalarE: sigmoid([i_j f_j] fused 512 cols), tanh(g_j), sigmoid(o_j),
  tanh(c_j), all PSUM->SBUF, bf16 outputs.
  VectorE/GpSimd: ig = i*g, fc = f*c (GpSimd), c = ig+fc, h = o*tanh(c),
  bf16 for DVE 2x mode.
  FC epilogue: out[100, BS] = W_fc @ h (+b_fc via ACT Identity bias);
  host transposes to [BS, 100].

Host side packs/permutes/casts the weights and pre-transposes messages
into x_rep [128, T*BS] bf16 (4 replicas of [V+1, t, b] at partition
offsets 0/32/64/96). All numerics on device; bf16 operand rounding gives
~5e-3 scale-relative absmax vs the fp32 reference.
"""

import numpy as np
import ml_dtypes

import concourse.bacc as bacc
import concourse.mybir as mybir
import concourse.tile as tile
from concourse.bass_utils import run_bass_kernel_spmd

F32 = mybir.dt.float32
BF16 = mybir.dt.bfloat16
AF = mybir.ActivationFunctionType

B, T, V = 2048, 128, 25
H = 512
NCLS = 100
CORES = 8
BS = B // CORES          # 256 batch rows per core
KT = H // 128            # 4 k-tiles (h chunks)
MT = (4 * H) // 128      # 16 m-tiles
NB = 8                   # psum banks


def _gate_perm():
    """Permutation of the 4H gate dim: m-tile m=4j+q -> gate q, h-chunk j."""
    idx = []
    for j in range(4):
        for base in (0, H, 2 * H, 3 * H):           # i, f, g, o
            idx.extend(range(base + j * 128, base + (j + 1) * 128))
    return np.array(idx)


def _pack_host(messages, W_ih, W_hh, b_ih, b_hh, W_fc, b_fc):
    perm = _gate_perm()
    W_hh_p = W_hh[perm]                              # [2048, 512]
    whh = np.zeros((128, KT * MT * 128), np.float32)
    for k in range(KT):
        for m in range(MT):
            t_ = W_hh_p[m * 128:(m + 1) * 128, k * 128:(k + 1) * 128].T
            whh[:, (k * MT + m) * 128:(k * MT + m + 1) * 128] = t_
    whh = whh.astype(ml_dtypes.bfloat16)

    W_ih_p = W_ih[perm]                              # [2048, 25]
    bias_p = (b_ih + b_hh)[perm]                     # [2048]
    wih_aug = np.zeros((26, 4 * H), np.float32)
    wih_aug[:25] = W_ih_p.T
    wih_aug[25] = bias_p
    wih = np.zeros((128, 4 * H), np.float32)
    for r in range(4):
        wih[r * 32:r * 32 + 26] = wih_aug
    wih = wih.astype(ml_dtypes.bfloat16)

    wfc = np.zeros((128, KT * NCLS), np.float32)
    W_fc_T = W_fc.T                                  # [512, 100]
    for k in range(KT):
        wfc[:, k * NCLS:(k + 1) * NCLS] = W_fc_T[k * 128:(k + 1) * 128]
    wfc = wfc.astype(ml_dtypes.bfloat16)

    bfc = b_fc.astype(np.float32).reshape(NCLS, 1)

    in_maps = []
    for c in range(CORES):
        shard = messages[c * BS:(c + 1) * BS]        # [BS, T, V]
        xT = np.ascontiguousarray(shard.transpose(2, 1, 0))  # [V, T, BS]
        x_rep = np.zeros((128, T, BS), np.float32)
        for r in range(4):
            x_rep[r * 32:r * 32 + 25] = xT
            x_rep[r * 32 + 25] = 1.0
        x_rep = x_rep.reshape(128, T * BS).astype(ml_dtypes.bfloat16)
        in_maps.append({"x_rep": x_rep, "whh": whh, "wih": wih,
                        "wfc": wfc, "bfc": bfc})
    return in_maps


def _build():
    nc = bacc.Bacc("TRN2", target_bir_lowering=False, debug=False)

    x_dram = nc.dram_tensor("x_rep", [128, T * BS], BF16,
                            kind="ExternalInput").ap()
    whh_dram = nc.dram_tensor("whh", [128, KT * MT * 128], BF16,
                              kind="ExternalInput").ap()
    wih_dram = nc.dram_tensor("wih", [128, 4 * H], BF16,
                              kind="ExternalInput").ap()
    wfc_dram = nc.dram_tensor("wfc", [128, KT * NCLS], BF16,
                              kind="ExternalInput").ap()
    bfc_dram = nc.dram_tensor("bfc", [NCLS, 1], F32,
                              kind="ExternalInput").ap()
    out_dram = nc.dram_tensor("out", [NCLS, BS], F32,
                              kind="ExternalOutput").ap()

    with tile.TileContext(nc) as tc:
        with (
            tc.tile_pool(name="const", bufs=1) as cpool,
            tc.tile_pool(name="xbuf", bufs=1) as xpool,
            tc.tile_pool(name="state", bufs=1) as spool,
            tc.tile_pool(name="psum", bufs=1, space="PSUM") as ppool,
            tc.tile_pool(name="work", bufs=3) as wpool,
        ):
            whh_sb = cpool.tile([128, KT * MT * 128], BF16)
            wih_sb = cpool.tile([128, 4 * H], BF16)
            wfc_sb = cpool.tile([128, KT * NCLS], BF16)
            bfc_sb = cpool.tile([NCLS, 1], F32)
            x_sb = xpool.tile([128, T * BS], BF16)
            h_sb = spool.tile([128, KT * BS], BF16)
            c_sb = spool.tile([128, KT * BS], BF16)

            nc.sync.dma_start(whh_sb[:], whh_dram[:])
            nc.sync.dma_start(wih_sb[:], wih_dram[:])
            nc.sync.dma_start(wfc_sb[:], wfc_dram[:])
            nc.sync.dma_start(bfc_sb[:], bfc_dram[:])
            xc = T * BS // 8
            for i in range(8):
                nc.sync.dma_start(x_sb[:, i * xc:(i + 1) * xc],
                                  x_dram[:, i * xc:(i + 1) * xc])

            nc.vector.memset(h_sb[:], 0.0)
            nc.vector.memset(c_sb[:], 0.0)

            gb = []
            for b_ in range(NB):
                t_ = ppool.tile([128, 512], F32, name=f"gbank{b_}")
                gb.append(t_)

            for t in range(T):
                xs = x_sb[:, t * BS:(t + 1) * BS]
                for phase in range(2):
                    for m in range(phase, MT, 2):
                        r = (m // 2) % 4
                        nc.tensor.matmul(
                            gb[m // 2][:, (m % 2) * BS:(m % 2 + 1) * BS],
                            wih_sb[r * 32:r * 32 + 26,
                                   m * 128:(m + 1) * 128],
                            xs[r * 32:r * 32 + 26, :],
                            start=(phase == 0), stop=False,
                            tile_position=(r * 32, 0),
                        )
                for k in range(KT):
                    for m in range(MT):
                        nc.tensor.matmul(
                            gb[m // 2][:, (m % 2) * BS:(m % 2 + 1) * BS],
                            whh_sb[:, (k * MT + m) * 128:
                                   (k * MT + m + 1) * 128],
                            h_sb[:, k * BS:(k + 1) * BS],
                            start=False,
                            stop=(k == KT - 1 and m % 2 == 1),
                        )
                for j in range(4):
                    if_t = wpool.tile([128, 512], BF16, tag="if")
                    g_t = wpool.tile([128, BS], BF16, tag="g")
                    o_t = wpool.tile([128, BS], BF16, tag="o")
                    ig_t = wpool.tile([128, BS], BF16, tag="ig")
                    fc_t = wpool.tile([128, BS], BF16, tag="fc")
                    tc_t = wpool.tile([128, BS], BF16, tag="tc")
                    nc.scalar.activation(if_t[:], gb[2 * j][:], AF.Sigmoid)
                    nc.scalar.activation(g_t[:], gb[2 * j + 1][:, 0:BS],
                                         AF.Tanh)
                    nc.scalar.activation(o_t[:], gb[2 * j + 1][:, BS:2 * BS],
                                         AF.Sigmoid)
                    cj = c_sb[:, j * BS:(j + 1) * BS]
                    nc.vector.tensor_mul(ig_t[:], if_t[:, 0:BS], g_t[:])
                    nc.gpsimd.tensor_mul(fc_t[:], if_t[:, BS:2 * BS], cj)
                    nc.vector.tensor_add(cj, ig_t[:], fc_t[:])
                    nc.scalar.activation(tc_t[:], cj, AF.Tanh)
                    nc.vector.tensor_mul(h_sb[:, j * BS:(j + 1) * BS],
                                         o_t[:], tc_t[:])

            for k in range(KT):
                nc.tensor.matmul(
                    gb[0][0:NCLS, 0:BS],
                    wfc_sb[:, k * NCLS:(k + 1) * NCLS],
                    h_sb[:, k * BS:(k + 1) * BS],
                    start=(k == 0), stop=(k == KT - 1),
                )
            out_sb = cpool.tile([NCLS, BS], F32)
            nc.scalar.activation(out_sb[:], gb[0][0:NCLS, 0:BS],
                                 AF.Identity, bias=bfc_sb[:])
            nc.sync.dma_start(out_dram[:], out_sb[:])

    nc.compile()
    return nc


_NC_CACHE = None


def kernel(messages, W_ih, W_hh, b_ih, b_hh, W_fc, b_fc):
    """Full-input entry point: shard, run on 8 NeuronCores, gather."""
    global _NC_CACHE
    messages = np.asarray(messages, np.float32)
    W_ih = np.asarray(W_ih, np.float32)
    W_hh = np.asarray(W_hh, np.float32)
    b_ih = np.asarray(b_ih, np.float32)
    b_hh = np.asarray(b_hh, np.float32)
    W_fc = np.asarray(W_fc, np.float32)
    b_fc = np.asarray(b_fc, np.float32)

    in_maps = _pack_host(messages, W_ih, W_hh, b_ih, b_hh, W_fc, b_fc)
    if _NC_CACHE is None:
        _NC_CACHE = _build()
    res = run_bass_kernel_spmd(_NC_CACHE, in_maps, list(range(CORES)))
    outs = [np.ascontiguousarray(np.asarray(res.results[c]["out"]).T)
            for c in range(CORES)]
    return np.concatenate(outs, axis=0).astype(np.float32)



# revision 13
# speedup vs baseline: 1.3971x; 1.3971x over previous
"""Trainium2 Bass kernel for nn_DiagnosticRNN (LSTM B=2048,T=128,V=25,H=512
-> FC 100), 8-way batch-data-parallel across NeuronCores.

Strategy (v3: hybrid fp8/bf16)
------------------------------
Data-parallel over batch: each of the 8 cores runs the full T=128 LSTM
recurrence on BS=256 batch rows with all weights replicated.

LSTM forget gates (mean ~0.73) decay perturbations geometrically, so fp8
quantization noise injected >16 steps before the end is invisible in the
final output. Steps 0..111 therefore run the whole matmul path in
fp8e4m3 with MatmulPerfMode.DoubleRow (2 K-planes per instruction at 0.5
cycles/output-column: W_hh sweep = 32 matmuls instead of 64, x-term = 16
thin-K matmuls with the 26 input channels split 13/13 across the two
planes), while the last 16 steps run the exact bf16 path of the v2
kernel. CPU-simulated end-to-end rel-err 0.0069 vs 0.0062 for all-bf16.

fp8 scaling: W_hh/W_ih_aug scaled x16 at quantization; h/x unscaled;
PSUM holds 16x the gate preactivations and the ScalarE activations apply
scale=1/16. c stays bf16 throughout; h is written as fp8 (DVE mul with
fp8 output) during fp8 steps and as bf16 at the phase boundary.

PSUM: four [128,1024] f32 tiles (2 banks each). fp8-phase layout puts
(i,f) / (g,o) pairs so ScalarE runs 8 wide activations per step
([128,1024] sigmoid over i0f0i1f1 etc.); the epilogue runs at j-pair
granularity, matching the DoubleRow pairing of h chunks, preserving the
cross-step PE/ACT/DVE pipeline. Elementwise work is split DVE (i*g,
c=ig+fc, h=o*tanh(c)) / GpSimd (f*c).
"""

import numpy as np
import ml_dtypes

import concourse.bacc as bacc
import concourse.mybir as mybir
import concourse.tile as tile
from concourse.bass_utils import run_bass_kernel_spmd

F32 = mybir.dt.float32
BF16 = mybir.dt.bfloat16
FP8 = mybir.dt.float8e4
AF = mybir.ActivationFunctionType
DR = mybir.MatmulPerfMode.DoubleRow

B, T, V = 2048, 128, 25
H = 512
NCLS = 100
CORES = 8
BS = B // CORES          # 256 batch rows per core
KT = H // 128            # 4 k-tiles (h chunks)
MT = (4 * H) // 128      # 16 m-tiles
T_TAIL = 16              # trailing bf16 steps
SW = 16.0                # fp8 weight quantization scale

E4NP = ml_dtypes.float8_e4m3
BFNP = ml_dtypes.bfloat16

# fp8-phase m-tile order (q=gate 0..3 -> i,f,g,o ; j=h chunk):
# psum col layout: [i0 f0 i1 f1 | i2 f2 i3 f3 | g0 g1 g2 g3 | o0 o1 o2 o3]
MLIST8 = [(0, 0), (1, 0), (0, 1), (1, 1), (0, 2), (1, 2), (0, 3), (1, 3),
          (2, 0), (2, 1), (2, 2), (2, 3), (3, 0), (3, 1), (3, 2), (3, 3)]

# fp8-step epilogue emission order: (op, j-pair). Engines run their own
# queues in emission order, so this IS the schedule.
# ACT: IF01/IF23/IFALL (sigmoid i,f), G01/G23/GALL (tanh), O01/O23/OALL
# (sigmoid), TC (tanh c). DVE: IG, CU, HM, FCD. Pool: FCP.
EPI_SCHED = [
    ("IF01", 0), ("G01", 0), ("FCP", 0), ("IG", 0), ("IF23", 1),
    ("CU", 0), ("G23", 1), ("FCD", 1), ("IG", 1), ("O01", 0),
    ("TC", 0), ("CU", 1), ("HM", 0), ("O23", 1), ("TC", 1), ("HM", 1),
]


def _rows8(m8):
    q, j = MLIST8[m8]
    return slice(q * H + j * 128, q * H + (j + 1) * 128)


def _gate_perm():
    """bf16-phase permutation: m-tile m=4j+q -> gate q, h-chunk j."""
    idx = []
    for j in range(4):
        for base in (0, H, 2 * H, 3 * H):           # i, f, g, o
            idx.extend(range(base + j * 128, base + (j + 1) * 128))
    return np.array(idx)


def pack_host(messages, W_ih, W_hh, b_ih, b_hh, W_fc, b_fc, T_steps=T):
    messages = np.asarray(messages, np.float32)
    W_ih = np.asarray(W_ih, np.float32)
    W_hh = np.asarray(W_hh, np.float32)
    b_ih = np.asarray(b_ih, np.float32)
    b_hh = np.asarray(b_hh, np.float32)
    W_fc = np.asarray(W_fc, np.float32)
    b_fc = np.asarray(b_fc, np.float32)
    t_tail = min(T_TAIL, T_steps)
    t8 = T_steps - t_tail

    # ---- bf16 tail weights (baseline layouts) ----
    perm = _gate_perm()
    W_hh_p = W_hh[perm]
    whh = np.zeros((128, KT * MT * 128), np.float32)
    for k in range(KT):
        for m in range(MT):
            t_ = W_hh_p[m * 128:(m + 1) * 128, k * 128:(k + 1) * 128].T
            whh[:, (k * MT + m) * 128:(k * MT + m + 1) * 128] = t_
    whh = whh.astype(BFNP)

    W_ih_p = W_ih[perm]
    bias_p = (b_ih + b_hh)[perm]
    wih_aug = np.zeros((26, 4 * H), np.float32)
    wih_aug[:25] = W_ih_p.T
    wih_aug[25] = bias_p
    wih = np.zeros((128, 4 * H), np.float32)
    for r in range(4):
        wih[r * 32:r * 32 + 26] = wih_aug
    wih = wih.astype(BFNP)

    wfc = np.zeros((128, KT * NCLS), np.float32)
    W_fc_T = W_fc.T
    for k in range(KT):
        wfc[:, k * NCLS:(k + 1) * NCLS] = W_fc_T[k * 128:(k + 1) * 128]
    wfc = wfc.astype(BFNP)
    bfc = b_fc.astype(np.float32).reshape(NCLS, 1)

    # ---- fp8 weights ----
    whh8 = np.zeros((128, 2 * MT, 2, 128), np.float32)
    for p in range(2):
        for pl in range(2):
            kc = slice((2 * p + pl) * 128, (2 * p + pl + 1) * 128)
            for m8 in range(MT):
                whh8[:, p * MT + m8, pl, :] = W_hh[_rows8(m8), kc].T * SW
    whh8 = whh8.astype(E4NP)

    waug = np.zeros((26, 4 * H), np.float32)
    waug[:25] = W_ih.T
    waug[25] = b_ih + b_hh
    wih8 = np.zeros((13, MT, 2, 128), np.float32)
    for pl in range(2):
        for m8 in range(MT):
            wih8[:, m8, pl, :] = waug[pl * 13:(pl + 1) * 13, _rows8(m8)] * SW
    wih8 = wih8.astype(E4NP)

    in_maps = []
    for c in range(CORES):
        shard = messages[c * BS:(c + 1) * BS]          # [BS, T, V]
        # fp8 x for steps [0, t8): [13, 2, t8*BS]
        xT = shard.transpose(2, 1, 0)                  # [V, T, BS]
        xaug = np.concatenate([xT, np.ones((1,) + xT.shape[1:], np.float32)])
        x8 = np.zeros((13, max(t8, 1), 2, BS), np.float32)
        if t8 > 0:
            x8[:, :t8, 0, :] = xaug[0:13, :t8]
            x8[:, :t8, 1, :] = xaug[13:26, :t8]
        x8 = x8.astype(E4NP)
        # bf16 x for tail steps: baseline 4-replica layout
        x_rep = np.zeros((128, max(t_tail, 1), BS), np.float32)
        if t_tail > 0:
            for r in range(4):
                x_rep[r * 32:r * 32 + 25, :] = xT[:, t8:t8 + t_tail]
                x_rep[r * 32 + 25] = 1.0
        x_rep = x_rep.reshape(128, -1).astype(BFNP)
        in_maps.append({"x8": x8, "x16": x_rep, "whh": whh, "wih": wih,
                        "whh8": whh8, "wih8": wih8, "wfc": wfc, "bfc": bfc})
    return in_maps


def build(T_steps=T, reps=1):
    t_tail = min(T_TAIL, T_steps)
    t8 = T_steps - t_tail
    nc = bacc.Bacc("TRN2", target_bir_lowering=False, debug=False)

    x8_dram = nc.dram_tensor("x8", [13, max(t8, 1), 2, BS], FP8,
                             kind="ExternalInput").ap()
    x16_dram = nc.dram_tensor("x16", [128, max(t_tail, 1) * BS], BF16,
                              kind="ExternalInput").ap()
    whh_dram = nc.dram_tensor("whh", [128, KT * MT * 128], BF16,
                              kind="ExternalInput").ap()
    wih_dram = nc.dram_tensor("wih", [128, 4 * H], BF16,
                              kind="ExternalInput").ap()
    whh8_dram = nc.dram_tensor("whh8", [128, 2 * MT, 2, 128], FP8,
                               kind="ExternalInput").ap()
    wih8_dram = nc.dram_tensor("wih8", [13, MT, 2, 128], FP8,
                               kind="ExternalInput").ap()
    wfc_dram = nc.dram_tensor("wfc", [128, KT * NCLS], BF16,
                              kind="ExternalInput").ap()
    bfc_dram = nc.dram_tensor("bfc", [NCLS, 1], F32,
                              kind="ExternalInput").ap()
    out_dram = nc.dram_tensor("out", [NCLS, BS], F32,
                              kind="ExternalOutput").ap()

    with tile.TileContext(nc) as tc:
        with (
            tc.tile_pool(name="const", bufs=1) as cpool,
            tc.tile_pool(name="xbuf", bufs=1) as xpool,
            tc.tile_pool(name="state", bufs=1) as spool,
            tc.tile_pool(name="psum", bufs=1, space="PSUM") as ppool,
            tc.tile_pool(name="work", bufs=3) as wpool,
        ):
            whh_sb = cpool.tile([128, KT * MT * 128], BF16)
            wih_sb = cpool.tile([128, 4 * H], BF16)
            whh8_sb = cpool.tile([128, 2 * MT, 2, 128], FP8)
            wih8_sb = cpool.tile([13, MT, 2, 128], FP8)
            wfc_sb = cpool.tile([128, KT * NCLS], BF16)
            bfc_sb = cpool.tile([NCLS, 1], F32)
            x8_sb = xpool.tile([13, max(t8, 1), 2, BS], FP8)
            x16_sb = xpool.tile([128, max(t_tail, 1) * BS], BF16)
            h_sb = spool.tile([128, KT * BS], BF16)
            c_sb = spool.tile([128, KT * BS], BF16)
            h8 = spool.tile([128, 2, 2, BS], FP8)

            nc.sync.dma_start(whh_sb[:], whh_dram[:])
            nc.sync.dma_start(wih_sb[:], wih_dram[:])
            nc.sync.dma_start(whh8_sb[:], whh8_dram[:])
            nc.sync.dma_start(wih8_sb[:], wih8_dram[:])
            nc.sync.dma_start(wfc_sb[:], wfc_dram[:])
            nc.sync.dma_start(bfc_sb[:], bfc_dram[:])
            if t8 > 0:
                th_ = t8 // 2
                nc.sync.dma_start(x8_sb[:, 0:th_], x8_dram[:, 0:th_])
                nc.sync.dma_start(x8_sb[:, th_:t8], x8_dram[:, th_:t8])
            if t_tail > 0:
                xc = t_tail * BS // 4
                for i in range(4):
                    nc.sync.dma_start(x16_sb[:, i * xc:(i + 1) * xc],
                                      x16_dram[:, i * xc:(i + 1) * xc])

            # PSUM: one tile spanning all 8 banks (range-tracked deps)
            PA = ppool.tile([128, 4096], F32)

            def p_slice8(m8):
                return PA[:, m8 * 256:(m8 + 1) * 256]

            def gb_sl(bank, lo, hi):
                """bf16-phase view: bank b (0..7) cols [lo:hi) (0..512)."""
                return PA[:, bank * 512 + lo:bank * 512 + hi]

            for rep in range(reps):
                nc.vector.memset(h_sb[:], 0.0)
                nc.vector.memset(c_sb[:], 0.0)
                nc.vector.memset(h8[:], 0.0)

                # ======== fp8 DoubleRow steps ========
                for t in range(t8):
                    xs = x8_sb[:, t, :, :]
                    for m8 in range(MT):
                        nc.tensor.matmul(
                            p_slice8(m8),
                            wih8_sb[:, m8, :, :],
                            xs, start=(m8 % 2 == 0), stop=False,
                            perf_mode=DR)
                    for p in range(2):
                        hs = h8[:, p, :, :]
                        for m8 in range(MT):
                            nc.tensor.matmul(
                                p_slice8(m8),
                                whh8_sb[:, p * MT + m8, :, :],
                                hs, start=False, stop=(p == 1),
                                perf_mode=DR)

                    last8 = (t == t8 - 1)
                    tl = {}
                    for nm, w_ in (("if_t", 2048), ("g_t", 1024),
                                   ("o_t", 1024), ("ig0", 512), ("ig1", 512),
                                   ("fc0", 512), ("fc1", 512),
                                   ("tc0", 512), ("tc1", 512)):
                        tl[nm] = wpool.tile([128, w_], BF16, tag=nm, name=nm)

                    def _if_out(lo, n_h):
                        # psum (h,j,s,n) -> sbuf addr s*1024 + h*512 + j*256+n
                        return tl["if_t"][:, lo:lo + n_h * 1024].rearrange(
                            "p (s h j n) -> p h j s n", s=2, h=n_h, j=2)

                    def IFALL(h_):
                        nc.scalar.activation(_if_out(0, 2), PA[:, 0:2048],
                                             AF.Sigmoid, scale=1.0 / SW)

                    def IF01(h_):
                        out = tl["if_t"][:].rearrange(
                            "p (s h j n) -> p h j s n", s=2, h=2, j=2
                        )[:, 0]
                        nc.scalar.activation(out, PA[:, 0:1024],
                                             AF.Sigmoid, scale=1.0 / SW)

                    def IF23(h_):
                        out = tl["if_t"][:].rearrange(
                            "p (s h j n) -> p h j s n", s=2, h=2, j=2
                        )[:, 1]
                        nc.scalar.activation(out, PA[:, 1024:2048],
                                             AF.Sigmoid, scale=1.0 / SW)

                    def GALL(h_):
                        nc.scalar.activation(tl["g_t"][:], PA[:, 2048:3072],
                                             AF.Tanh, scale=1.0 / SW)

                    def G01(h_):
                        nc.scalar.activation(tl["g_t"][:, 0:512],
                                             PA[:, 2048:2560],
                                             AF.Tanh, scale=1.0 / SW)

                    def G23(h_):
                        nc.scalar.activation(tl["g_t"][:, 512:1024],
                                             PA[:, 2560:3072],
                                             AF.Tanh, scale=1.0 / SW)

                    def OALL(h_):
                        nc.scalar.activation(tl["o_t"][:], PA[:, 3072:4096],
                                             AF.Sigmoid, scale=1.0 / SW)

                    def O01(h_):
                        nc.scalar.activation(tl["o_t"][:, 0:512],
                                             PA[:, 3072:3584],
                                             AF.Sigmoid, scale=1.0 / SW)

                    def O23(h_):
                        nc.scalar.activation(tl["o_t"][:, 512:1024],
                                             PA[:, 3584:4096],
                                             AF.Sigmoid, scale=1.0 / SW)

                    def TC(h_):
                        nc.scalar.activation(tl[f"tc{h_}"][:],
                                             c_sb[:, h_ * 512:(h_ + 1) * 512],
                                             AF.Tanh)

                    def IG(h_):
                        nc.vector.tensor_mul(
                            tl[f"ig{h_}"][:],
                            tl["if_t"][:, h_ * 512:(h_ + 1) * 512],
                            tl["g_t"][:, h_ * 512:(h_ + 1) * 512])

                    def FCP(h_):
                        nc.gpsimd.tensor_mul(
                            tl[f"fc{h_}"][:],
                            tl["if_t"][:, 1024 + h_ * 512:1024 + (h_ + 1) * 512],
                            c_sb[:, h_ * 512:(h_ + 1) * 512])

                    def FCD(h_):
                        nc.vector.tensor_mul(
                            tl[f"fc{h_}"][:],
                            tl["if_t"][:, 1024 + h_ * 512:1024 + (h_ + 1) * 512],
                            c_sb[:, h_ * 512:(h_ + 1) * 512])

                    def CU(h_):
                        nc.vector.tensor_add(
                            c_sb[:, h_ * 512:(h_ + 1) * 512],
                            tl[f"ig{h_}"][:], tl[f"fc{h_}"][:])

                    def HM(h_):
                        nc.vector.tensor_mul(
                            h8[:, h_, :, :],
                            tl["o_t"][:, h_ * 512:(h_ + 1) * 512],
                            tl[f"tc{h_}"][:])
                        if last8 and t_tail > 0:
                            nc.vector.tensor_mul(
                                h_sb[:, h_ * 512:(h_ + 1) * 512],
                                tl["o_t"][:, h_ * 512:(h_ + 1) * 512],
                                tl[f"tc{h_}"][:])

                    ops = {"IFALL": IFALL, "IF01": IF01, "IF23": IF23,
                           "GALL": GALL, "G01": G01, "G23": G23,
                           "OALL": OALL, "O01": O01, "O23": O23,
                           "TC": TC, "IG": IG, "FCP": FCP, "FCD": FCD,
                           "CU": CU, "HM": HM}
                    for name, h_ in EPI_SCHED:
                        ops[name](h_)

                # ======== bf16 tail steps (baseline structure) ========
                for tt in range(t_tail):
                    xs = x16_sb[:, tt * BS:(tt + 1) * BS]
                    for phase in range(2):
                        for m in range(phase, MT, 2):
                            r = (m // 2) % 4
                            nc.tensor.matmul(
                                gb_sl(m // 2, (m % 2) * BS, (m % 2 + 1) * BS),
                                wih_sb[r * 32:r * 32 + 26,
                                       m * 128:(m + 1) * 128],
                                xs[r * 32:r * 32 + 26, :],
                                start=(phase == 0), stop=False,
                                tile_position=(r * 32, 0),
                            )
                    for k in range(KT):
                        for m in range(MT):
                            nc.tensor.matmul(
                                gb_sl(m // 2, (m % 2) * BS, (m % 2 + 1) * BS),
                                whh_sb[:, (k * MT + m) * 128:
                                       (k * MT + m + 1) * 128],
                                h_sb[:, k * BS:(k + 1) * BS],
                                start=False,
                                stop=(k == KT - 1 and m % 2 == 1),
                            )
                    for j in range(4):
                        if_t = wpool.tile([128, 512], BF16, tag="tif")
                        g_t = wpool.tile([128, BS], BF16, tag="tg")
                        o_t = wpool.tile([128, BS], BF16, tag="to")
                        ig_t = wpool.tile([128, BS], BF16, tag="tig")
                        fc_t = wpool.tile([128, BS], BF16, tag="tfc")
                        tc_t = wpool.tile([128, BS], BF16, tag="ttc")
                        nc.scalar.activation(if_t[:], gb_sl(2 * j, 0, 512),
                                             AF.Sigmoid)
                        nc.scalar.activation(g_t[:], gb_sl(2 * j + 1, 0, BS),
                                             AF.Tanh)
                        nc.scalar.activation(o_t[:],
                                             gb_sl(2 * j + 1, BS, 2 * BS),
                                             AF.Sigmoid)
                        cj = c_sb[:, j * BS:(j + 1) * BS]
                        nc.vector.tensor_mul(ig_t[:], if_t[:, 0:BS], g_t[:])
                        nc.gpsimd.tensor_mul(fc_t[:], if_t[:, BS:2 * BS], cj)
                        nc.vector.tensor_add(cj, ig_t[:], fc_t[:])
                        nc.scalar.activation(tc_t[:], cj, AF.Tanh)
                        nc.vector.tensor_mul(h_sb[:, j * BS:(j + 1) * BS],
                                             o_t[:], tc_t[:])

            for k in range(KT):
                nc.tensor.matmul(
                    PA[0:NCLS, 0:BS],
                    wfc_sb[:, k * NCLS:(k + 1) * NCLS],
                    h_sb[:, k * BS:(k + 1) * BS],
                    start=(k == 0), stop=(k == KT - 1),
                )
            out_sb = cpool.tile([NCLS, BS], F32)
            nc.scalar.activation(out_sb[:], PA[0:NCLS, 0:BS],
                                 AF.Identity, bias=bfc_sb[:])
            nc.sync.dma_start(out_dram[:], out_sb[:])

    nc.compile()
    return nc


_NC_CACHE = None


def kernel(messages, W_ih, W_hh, b_ih, b_hh, W_fc, b_fc):
    """Full-input entry point: shard, run on 8 NeuronCores, gather."""
    global _NC_CACHE
    in_maps = pack_host(messages, W_ih, W_hh, b_ih, b_hh, W_fc, b_fc, T)
    if _NC_CACHE is None:
        _NC_CACHE = build(T)
    res = run_bass_kernel_spmd(_NC_CACHE, in_maps, list(range(CORES)))
    outs = [np.ascontiguousarray(np.asarray(res.results[c]["out"]).T)
            for c in range(CORES)]
    return np.concatenate(outs, axis=0).astype(np.float32)
